# revision 1
# baseline (speedup 1.0000x reference)
"""CosineAttention Trainium2 kernel (8-core SPMD, head-sharded).

Sharding: core c handles heads {2c, 2c+1} for both batches.
Per-core device program (identical across cores; data differs):
  Phase A: qT/kT projected transposed ([d,2h]-part x tok-free), l2-normalized
           via PE block-ones matmul + K=2 broadcast matmul; v projected in
           natural [tok, d] layout with an extra ones column for the softmax
           denominator.
  Phase B: dots^T = khat^T q (2-head row-packed, K=64 concurrent pairs);
           (dots*temp + pos_biasT) on DVE in one scalar_tensor_tensor;
           exp on ACT; attn@v with [v|1] stationary -> out^T rows + Z row;
           Z-normalize via K=1 broadcast matmul + DVE mul.
  Phase C: out^T @ W_out block -> per-core partial [B, N, C]; host sums.
"""

import sys

sys.path.insert(0, "/opt/trn_rl_repo")

import numpy as np
import ml_dtypes

import concourse.bass as bass
import concourse.bacc as bacc
import concourse.tile as tile
from concourse import mybir
from concourse import bass_utils

F32 = mybir.dt.float32
BF16 = mybir.dt.bfloat16
AF = mybir.ActivationFunctionType
ALU = mybir.AluOpType

B, N, C, H, D = 2, 2048, 1024, 16, 64
NCORES = 8
HL = 2  # heads per core


def build_nc(temp: float, n: int = N, b_sz: int = B):
    """Emit the per-core program. Parameterized by sequence length for sim."""
    nc = bacc.Bacc("TRN2", target_bir_lowering=False)
    CT = C // 128            # contraction tiles for projections
    TBW = min(512, n)        # qk-proj token block width
    NTB = n // TBW
    KT = n // 128            # key tiles
    NH = n // 2              # q-half width (pos_bias SBUF residency unit)
    QW = min(512, NH)        # q block width
    NQB = NH // QW
    NCB = C // 512           # out-proj column blocks

    xt = nc.dram_tensor("xt", [b_sz, C, n], F32, kind="ExternalInput")
    wq = nc.dram_tensor("wq", [C, 128], F32, kind="ExternalInput")
    wk = nc.dram_tensor("wk", [C, 128], F32, kind="ExternalInput")
    wv = nc.dram_tensor("wv", [C, 128], F32, kind="ExternalInput")
    wo = nc.dram_tensor("wo", [128, C], F32, kind="ExternalInput")
    biasT = nc.dram_tensor("biasT", [HL, n, n], BF16, kind="ExternalInput")
    cbc = nc.dram_tensor("cbc", [2, 128], F32, kind="ExternalInput")
    out_p = nc.dram_tensor("out_p", [b_sz, n, C], F32, kind="ExternalOutput")

    with tile.TileContext(nc) as tc:
        with (
            tc.tile_pool(name="const", bufs=1) as cpool,
            tc.tile_pool(name="weights", bufs=1) as wpool,
            tc.tile_pool(name="qkvp", bufs=1) as qpool,
        ):
            # constants
            ones_bd = cpool.tile([128, 2], F32)       # block-diag head-sum
            nc.vector.memset(ones_bd[:], 0.0)
            nc.vector.memset(ones_bd[0:64, 0:1], 1.0)
            nc.vector.memset(ones_bd[64:128, 1:2], 1.0)
            ones2t = cpool.tile([128, 128], F32)      # per-head broadcast
            nc.sync.dma_start(ones2t[0:2, :], cbc[:])
            ones64 = cpool.tile([128, 64], F32)       # K=1 Z broadcast rows
            nc.vector.memset(ones64[:], 1.0)

            # weights
            wq_sb = wpool.tile([128, CT, 128], F32)
            wk_sb = wpool.tile([128, CT, 128], F32)
            wv_sb = wpool.tile([128, CT, 128], F32)
            nc.sync.dma_start(wq_sb[:], wq[:].rearrange("(ct p) j -> p ct j", p=128))
            nc.sync.dma_start(wk_sb[:], wk[:].rearrange("(ct p) j -> p ct j", p=128))
            nc.sync.dma_start(wv_sb[:], wv[:].rearrange("(ct p) j -> p ct j", p=128))
            wo_sb = wpool.tile([128, C], F32)
            nc.sync.dma_start(wo_sb[:], wo[:])

            # persistent per-batch activations
            qhat = [qpool.tile([128, n], F32, tag=f"qhat{b}", name=f"qhat{b}") for b in range(b_sz)]
            khat = [qpool.tile([128, n], F32, tag=f"khat{b}", name=f"khat{b}") for b in range(b_sz)]
            # v layout per kt: [0:64]=v_h0 | [64]=1 | [98]=1 | [130:194]=v_h1
            # h0 stationary = cols 0:65 (M=65, Z at out row 64)
            # h1 stationary = cols 66:194 (M=128, Z at out row 32, v at 64:128)
            vsb = [qpool.tile([128, KT, 194], F32, tag=f"v{b}", name=f"v{b}") for b in range(b_sz)]
            outT = [qpool.tile([128, n], F32, tag=f"outT{b}", name=f"outT{b}") for b in range(b_sz)]
            for b in range(b_sz):
                nc.gpsimd.memset(vsb[b][:, :, 64:66], 1.0)
                nc.gpsimd.memset(vsb[b][:, :, 98:99], 1.0)
                # zero the junk windows read by the h1 stationary so HW
                # leftovers can't produce NaN*0 traps in unread psum rows
                nc.gpsimd.memset(vsb[b][:, :, 66:98], 0.0)
                nc.gpsimd.memset(vsb[b][:, :, 99:130], 0.0)

            # ---------------- Phase A: projections + l2 norm ----------------
            with (
                tc.tile_pool(name="xa", bufs=2) as xa,
                tc.tile_pool(name="pa_sb", bufs=4) as pasb,
                tc.tile_pool(name="ppq", bufs=2, space="PSUM") as ppq,
                tc.tile_pool(name="ppk", bufs=2, space="PSUM") as ppk,
                tc.tile_pool(name="ppv", bufs=2, space="PSUM") as ppv,
                tc.tile_pool(name="ppn2", bufs=1, space="PSUM") as ppn2,
                tc.tile_pool(name="pprbc", bufs=1, space="PSUM") as pprbc,
            ):
                for b in range(b_sz):
                    for tb in range(NTB):
                        tc0 = tb * TBW
                        xts = []
                        for ct in range(CT):
                            t = xa.tile([128, TBW], F32, tag=f"x{ct}", name=f"x{ct}")
                            nc.sync.dma_start(
                                t[:], xt[b, ct * 128:(ct + 1) * 128, tc0:tc0 + TBW]
                            )
                            xts.append(t)
                        for which, wsb, dst in (("q", wq_sb, qhat), ("k", wk_sb, khat)):
                            pp = ppq if which == "q" else ppk
                            pq = pp.tile([128, TBW], F32)
                            for ct in range(CT):
                                nc.tensor.matmul(
                                    pq[:], wsb[:, ct, :], xts[ct][:],
                                    start=(ct == 0), stop=(ct == CT - 1),
                                )
                            sq = pasb.tile([128, TBW], F32, tag="sq")
                            nc.scalar.square(sq[:], pq[:])
                            pn2 = ppn2.tile([128, TBW], F32)
                            nc.tensor.matmul(pn2[0:2, :], ones_bd[:, 0:2], sq[:])
                            nrm = pasb.tile([128, TBW], F32, tag="nrm")
                            nc.scalar.sqrt(nrm[0:2, :], pn2[0:2, :])
                            rec = pasb.tile([128, TBW], F32, tag="rec")
                            nc.vector.reciprocal(rec[0:2, :], nrm[0:2, :])
                            # r = min(1/||.||, 1/eps)  (== 1/max(||.||, eps))
                            nc.vector.tensor_scalar_min(rec[0:2, :], rec[0:2, :], 1e12)
                            prb = pprbc.tile([128, TBW], F32)
                            nc.tensor.matmul(prb[:], ones2t[0:2, :], rec[0:2, :])
                            rbc = pasb.tile([128, TBW], F32, tag="rbc")
                            nc.scalar.copy(rbc[:], prb[:])
                            nc.vector.tensor_mul(
                                dst[b][:, tc0:tc0 + TBW], pq[:], rbc[:]
                            )
                        for tl in range(TBW // 128):
                            kt = (tc0 // 128) + tl
                            pv = ppv.tile([128, 128], F32)
                            for ct in range(CT):
                                nc.tensor.matmul(
                                    pv[:], xts[ct][:, tl * 128:(tl + 1) * 128],
                                    wv_sb[:, ct, :],
                                    start=(ct == 0), stop=(ct == CT - 1),
                                )
                            nc.vector.tensor_copy(vsb[b][:, kt, 0:64], pv[:, 0:64])
                            nc.vector.tensor_copy(vsb[b][:, kt, 130:194], pv[:, 64:128])

            # ---------------- Phase B: attention ----------------
            with (
                tc.tile_pool(name="biasb", bufs=1) as bpool,
                tc.tile_pool(name="tsb", bufs=3) as tpool,
                tc.tile_pool(name="esb", bufs=3) as epool,
                tc.tile_pool(name="zsb", bufs=2) as zpool,
                tc.tile_pool(name="ppd", bufs=2, space="PSUM") as ppd,
                tc.tile_pool(name="ppo", bufs=1, space="PSUM") as ppo,
                tc.tile_pool(name="ppz", bufs=2, space="PSUM") as ppz,
            ):
                for qh in range(2):
                    btiles = []
                    for kt in range(KT):
                        t = bpool.tile([128, HL, NH], BF16, tag=f"bias{kt}", name=f"bias{kt}")
                        for h in range(HL):
                            nc.sync.dma_start(
                                t[:, h, :],
                                biasT[h, kt * 128:(kt + 1) * 128, qh * NH:(qh + 1) * NH],
                            )
                        btiles.append(t)
                    for b in range(b_sz):
                        for qb in range(NQB):
                            qc = qh * NH + qb * QW
                            po_a = ppo.tile([128, QW], F32, tag="poa")
                            po_b = ppo.tile([128, QW], F32, tag="pob")
                            for kt in range(KT):
                                pd = ppd.tile([128, 2, QW], F32)
                                nc.tensor.matmul(
                                    pd[:, 0, :],
                                    khat[b][0:64, kt * 128:(kt + 1) * 128],
                                    qhat[b][0:64, qc:qc + QW],
                                )
                                nc.tensor.matmul(
                                    pd[:, 1, :],
                                    khat[b][64:128, kt * 128:(kt + 1) * 128],
                                    qhat[b][64:128, qc:qc + QW],
                                )
                                ts = tpool.tile([128, 2, QW], F32)
                                nc.vector.scalar_tensor_tensor(
                                    ts[:], pd[:], temp,
                                    btiles[kt][:, :, qb * QW:qb * QW + QW],
                                    op0=ALU.mult, op1=ALU.add,
                                )
                                et = epool.tile([128, 2, QW], F32)
                                nc.scalar.activation(et[:], ts[:], AF.Exp)
                                nc.tensor.matmul(
                                    po_a[0:65, :], vsb[b][:, kt, 0:65], et[:, 0, :],
                                    start=(kt == 0), stop=(kt == KT - 1),
                                )
                                nc.tensor.matmul(
                                    po_b[:, :], vsb[b][:, kt, 66:194], et[:, 1, :],
                                    start=(kt == 0), stop=(kt == KT - 1),
                                )
                            zr = zpool.tile([128, QW], F32)
                            # h0: Z on psum row 64
                            nc.vector.reciprocal(zr[64:65, :], po_a[64:65, :])
                            pza = ppz.tile([128, QW], F32, tag="pza")
                            nc.tensor.matmul(
                                pza[0:64, :], ones64[64:65, 0:64], zr[64:65, :],
                                tile_position=(64, 0),
                            )
                            zba = zpool.tile([128, QW], F32, tag="zb")
                            nc.scalar.copy(zba[0:64, :], pza[0:64, :])
                            nc.vector.tensor_mul(
                                outT[b][0:64, qc:qc + QW], po_a[0:64, :], zba[0:64, :]
                            )
                            # h1: Z on psum row 32, out rows 64:128
                            nc.vector.reciprocal(zr[32:33, :], po_b[32:33, :])
                            pzb = ppz.tile([128, QW], F32, tag="pza")
                            nc.tensor.matmul(
                                pzb[64:128, :], ones64[32:33, 0:64], zr[32:33, :],
                                tile_position=(32, 64),
                            )
                            zbb = zpool.tile([128, QW], F32, tag="zb")
                            nc.scalar.copy(zbb[64:128, :], pzb[64:128, :])
                            nc.vector.tensor_mul(
                                outT[b][64:128, qc:qc + QW],
                                po_b[64:128, :], zbb[64:128, :],
                            )

            # ---------------- Phase C: output projection ----------------
            with (
                tc.tile_pool(name="osb", bufs=3) as opool,
                tc.tile_pool(name="ppc", bufs=2, space="PSUM") as ppc,
            ):
                for b in range(b_sz):
                    for tt in range(n // 128):
                        for cb in range(NCB):
                            pc = ppc.tile([128, 512], F32)
                            nc.tensor.matmul(
                                pc[:], outT[b][:, tt * 128:(tt + 1) * 128],
                                wo_sb[:, cb * 512:(cb + 1) * 512],
                            )
                            ob = opool.tile([128, 512], F32)
                            nc.scalar.copy(ob[:], pc[:])
                            nc.sync.dma_start(
                                out_p[b, tt * 128:(tt + 1) * 128,
                                      cb * 512:(cb + 1) * 512],
                                ob[:],
                            )
    nc.compile()
    return nc


def make_core_inputs(x, W_qkv, W_out, pos_bias, core: int):
    """Host-side shard prep for one core."""
    n = x.shape[1]
    xT = np.ascontiguousarray(np.transpose(x, (0, 2, 1)), dtype=np.float32)
    w4 = W_qkv.reshape(C, -1, D, 3)  # [C, H, D, 3]
    h0 = HL * core
    wq_c = np.ascontiguousarray(w4[:, h0:h0 + HL, :, 0].reshape(C, 128), np.float32)
    wk_c = np.ascontiguousarray(w4[:, h0:h0 + HL, :, 1].reshape(C, 128), np.float32)
    wv_c = np.ascontiguousarray(w4[:, h0:h0 + HL, :, 2].reshape(C, 128), np.float32)
    wo_c = np.ascontiguousarray(W_out[128 * core:128 * (core + 1), :], np.float32)
    bT = np.ascontiguousarray(
        np.transpose(pos_bias[h0:h0 + HL], (0, 2, 1))
    ).astype(ml_dtypes.bfloat16)
    cbc = np.zeros((2, 128), np.float32)
    cbc[0, 0:64] = 1.0
    cbc[1, 64:128] = 1.0
    return {"xt": xT, "wq": wq_c, "wk": wk_c, "wv": wv_c, "wo": wo_c,
            "biasT": bT, "cbc": cbc}


def _ref_numpy(x, W_qkv, W_out, temperature, pos_bias, mask):
    """Slow fallback (masked inputs); mirrors the jax reference."""
    b, n, c = x.shape
    qkv = (x @ W_qkv).reshape(b, n, H, D, 3)
    q = np.transpose(qkv[..., 0], (0, 2, 1, 3)).astype(np.float64)
    k = np.transpose(qkv[..., 1], (0, 2, 1, 3)).astype(np.float64)
    v = np.transpose(qkv[..., 2], (0, 2, 1, 3)).astype(np.float64)

    def l2n(t):
        nn = np.sqrt((t * t).sum(-1, keepdims=True))
        return t / np.maximum(nn, 1e-12)

    q, k = l2n(q), l2n(k)
    dots = np.einsum("bhid,bhjd->bhij", q, k) * float(temperature)
    dots = dots + pos_bias[None].astype(np.float64)
    valid = ~mask
    allowed = valid[:, None, :, None] & valid[:, None, None, :]
    dots = np.where(allowed, dots, -np.finfo(np.float32).max)
    dots = dots - dots.max(-1, keepdims=True)
    e = np.exp(dots)
    attn = e / e.sum(-1, keepdims=True)
    out = np.einsum("bhij,bhjd->bhid", attn, v)
    out = np.transpose(out, (0, 2, 1, 3)).reshape(b, n, H * D)
    return (out @ W_out.astype(np.float64)).astype(np.float32)


_NC_CACHE = {}


def kernel(x, W_qkv, W_out, temperature, pos_bias, mask):
    x = np.asarray(x, np.float32)
    W_qkv = np.asarray(W_qkv, np.float32)
    W_out = np.asarray(W_out, np.float32)
    pos_bias = np.asarray(pos_bias, np.float32)
    mask = np.asarray(mask)
    temp = float(np.asarray(temperature))
    if mask.any():
        return _ref_numpy(x, W_qkv, W_out, temp, pos_bias, mask)

    key = (temp, x.shape[1], x.shape[0])
    if key not in _NC_CACHE:
        _NC_CACHE[key] = build_nc(temp, n=x.shape[1], b_sz=x.shape[0])
    nc = _NC_CACHE[key]
    in_maps = [make_core_inputs(x, W_qkv, W_out, pos_bias, c) for c in range(NCORES)]
    res = bass_utils.run_bass_kernel_spmd(nc, in_maps, core_ids=list(range(NCORES)))
    out = np.zeros((x.shape[0], x.shape[1], C), np.float64)
    for r in res.results:
        out += r["out_p"].astype(np.float64)
    return out.astype(np.float32)

